# revision 3
# baseline (speedup 1.0000x reference)
"""VQ codebook (nearest-codebook-entry) kernel for Trainium2, 8 NeuronCores.

Problem: x (64, 64, 32, 32) f32, codebook (512, 64) f32.
  flat = x.transpose -> (b, n=1024, d=64) tokens
  idx = argmin_k ||flat - codebook[k]||^2   (first-min, as jnp.argmin)
  out = codebook[idx] transposed back to (64, 64, 32, 32)

Sharding: data-parallel over batch (8 batches per core x 8 cores), codebook
replicated.

Per-core pipeline (64 token-tiles of 128 tokens):
  PE:  score[tok,k] = x_aug.T @ cb_aug   (aug row folds -|e|^2/2; argmax score
       == argmin dist since ||x||^2 is constant per token)
  ACT: copy PSUM -> SBUF
  DVE: Max8 + MaxIndex -> idx (exact fp32, first-occurrence = argmin-first)
  GPSIMD: indirect DMA gather codebook rows by idx
  PE:  pair transpose [128tok,2x64d] -> [128d, 128tok]
  DMA: store to out[b, :, tok_slice]
"""

import numpy as np

import concourse.bass as bass
import concourse.mybir as mybir
import concourse.tile as tile
from concourse.bass_utils import run_bass_kernel_spmd
from concourse.masks import make_identity

N_CORES = 8
B_FULL, D, H, W = 64, 64, 32, 32
N = H * W          # 1024 tokens per batch
K = 512            # codebook entries
B_CORE = B_FULL // N_CORES   # 8 batches per core
TOK_TILE = 128
TILES_PER_BATCH = N // TOK_TILE          # 8
N_TILES = B_CORE * TILES_PER_BATCH       # 64 per core

F32 = mybir.dt.float32
U32 = mybir.dt.uint32

MAX_SYNC_WAITS = 1


def _split_excess_waits(nc, max_waits=MAX_SYNC_WAITS):
    """This container's walrus rejects >2 sync waits per instruction; move
    excess waits onto InstNoOp instructions inserted just before."""
    for f in nc.m.functions:
        for bb in f.blocks:
            new_list = []
            for inst in bb.instructions:
                si = inst.sync_info
                if si is not None and si.on_wait and len(si.on_wait) > max_waits:
                    waits = list(si.on_wait)
                    extra, keep = waits[:-max_waits], waits[-max_waits:]
                    for i, w in enumerate(extra):
                        nop = mybir.InstNoOp(name=f"{inst.name}-sw{i}", ins=[], outs=[])
                        nop.engine = inst.engine
                        nop.sync_info = mybir.SyncInfo(on_wait=[w], on_update=[])
                        new_list.append(nop)
                    si.on_wait = keep
                new_list.append(inst)
            bb.instructions[:] = new_list


def build_nc(reps=1):
    from contextlib import nullcontext

    nc = bass.Bass()
    x_aug = nc.dram_tensor("x_aug", [B_CORE, D + 1, N], F32, kind="ExternalInput")
    cb_aug = nc.dram_tensor("cb_aug", [D + 1, K], F32, kind="ExternalInput")
    cb = nc.dram_tensor("cb", [K, D], F32, kind="ExternalInput")
    out = nc.dram_tensor("out", [B_CORE, D, N], F32, kind="ExternalOutput")

    with tile.TileContext(nc) as tc:
        with (
            tc.tile_pool(name="const", bufs=1) as constp,
            tc.tile_pool(name="sbuf", bufs=3) as sb,
            tc.tile_pool(name="score", bufs=3) as scp,
            tc.tile_pool(name="pair", bufs=2) as pairp,
            tc.tile_pool(name="psum", bufs=4, space="PSUM") as ps,
            tc.tile_pool(name="psumt", bufs=2, space="PSUM") as pst,
        ):
            ident = constp.tile([128, 128], F32)
            make_identity(nc, ident[:])
            cba = constp.tile([D + 1, K], F32)
            nc.sync.dma_start(out=cba[:], in_=cb_aug[:])

            for _rep in range(reps):
                _emit_body(nc, sb, scp, pairp, ps, pst, x_aug, cb, out, cba, ident)

    _split_excess_waits(nc)
    return nc


def _emit_body(nc, sb, scp, pairp, ps, pst, x_aug, cb, out, cba, ident):
    pr = None
    pr_meta = []
    if True:
            for t in range(N_TILES):
                b, j = divmod(t, TILES_PER_BATCH)
                sl = slice(j * TOK_TILE, (j + 1) * TOK_TILE)

                xt = sb.tile([D + 1, TOK_TILE], F32, tag="xt")
                nc.sync.dma_start(out=xt[:], in_=x_aug[b, :, sl])

                score_ps = ps.tile([TOK_TILE, K], F32, tag="score_ps")
                nc.tensor.matmul(score_ps[:], lhsT=xt[:], rhs=cba[:],
                                 start=True, stop=True)

                score = scp.tile([TOK_TILE, K], F32, tag="score")
                nc.scalar.copy(score[:], score_ps[:])

                mx8 = sb.tile([TOK_TILE, 8], F32, tag="mx8")
                nc.vector.max(mx8[:], score[:])
                idx8 = sb.tile([TOK_TILE, 8], U32, tag="idx8")
                nc.vector.max_index(idx8[:], mx8[:], score[:])

                if t % 2 == 0:
                    pr = pairp.tile([TOK_TILE, 128], F32, tag="pair")
                    pr_meta = []
                pr_meta.append((b, sl))
                half = pr[:, (t % 2) * D:(t % 2 + 1) * D]
                nc.gpsimd.indirect_dma_start(
                    out=half,
                    out_offset=None,
                    in_=cb[:],
                    in_offset=bass.IndirectOffsetOnAxis(ap=idx8[:, 0:1], axis=0),
                )

                if t % 2 == 1:
                    tp = pst.tile([128, TOK_TILE], F32, tag="tp")
                    nc.tensor.transpose(tp[:], pr[:], ident[:])
                    tps = sb.tile([128, TOK_TILE], F32, tag="tps")
                    nc.scalar.copy(tps[:], tp[:])
                    for h, (bb_, sl_) in enumerate(pr_meta):
                        nc.sync.dma_start(
                            out=out[bb_, :, sl_],
                            in_=tps[h * D:(h + 1) * D, :],
                        )


_NC_CACHE = None


def _get_nc():
    global _NC_CACHE
    if _NC_CACHE is None:
        _NC_CACHE = build_nc()
    return _NC_CACHE


def prep_inputs(x, codebook):
    """Host-side prep: shard x over batch, augment with a ones-row; build
    cb_aug = [codebook.T ; -|e|^2/2]."""
    x = np.asarray(x, dtype=np.float32).reshape(B_FULL, D, N)
    codebook = np.asarray(codebook, dtype=np.float32)
    e2 = (codebook.astype(np.float64) ** 2).sum(-1).astype(np.float32)
    cb_aug = np.concatenate(
        [codebook.T, (-0.5 * e2)[None, :]], axis=0
    ).astype(np.float32)
    cb_aug = np.ascontiguousarray(cb_aug)
    cb = np.ascontiguousarray(codebook)

    in_maps = []
    for c in range(N_CORES):
        xs = x[c * B_CORE:(c + 1) * B_CORE]                      # [8, 64, 1024]
        ones = np.ones((B_CORE, 1, N), dtype=np.float32)
        x_aug = np.ascontiguousarray(np.concatenate([xs, ones], axis=1))
        in_maps.append({"x_aug": x_aug, "cb_aug": cb_aug, "cb": cb})
    return in_maps


def kernel(x, codebook):
    nc = _get_nc()
    in_maps = prep_inputs(x, codebook)
    res = run_bass_kernel_spmd(nc, in_maps, core_ids=list(range(N_CORES)))
    out = np.concatenate([r["out"] for r in res.results], axis=0)
    return out.reshape(B_FULL, D, H, W)


if __name__ == "__main__":
    rng = np.random.default_rng(0)
    x = rng.standard_normal((B_FULL, D, H, W)).astype(np.float32)
    cbk = rng.standard_normal((K, D)).astype(np.float32)
    got = kernel(x, cbk)
    # numpy reference
    flat = x.reshape(B_FULL, D, N).transpose(0, 2, 1)
    x2 = (flat ** 2).sum(-1, keepdims=True)
    e2 = (cbk ** 2).sum(-1)
    dist = x2 - 2.0 * flat @ cbk.T + e2
    idx = dist.argmin(-1)
    exp = cbk[idx].transpose(0, 2, 1).reshape(B_FULL, D, H, W)
    err = np.linalg.norm(got - exp) / np.linalg.norm(exp)
    print("rel err vs numpy:", err)
    nflip = (got != exp).any(axis=1).sum()
    print("mismatched token count:", int(nflip))


# revision 4
# speedup vs baseline: 2.8669x; 2.8669x over previous
"""VQ codebook (nearest-codebook-entry) kernel for Trainium2, 8 NeuronCores.

Problem: x (64, 64, 32, 32) f32, codebook (512, 64) f32.
  flat = x.transpose -> (b, n=1024, d=64) tokens
  idx = argmin_k ||flat - codebook[k]||^2   (first-min, as jnp.argmin)
  out = codebook[idx] transposed back to (64, 64, 32, 32)

Sharding: data-parallel over batch (8 batches per core x 8 cores), codebook
replicated.

Per-core pipeline (64 token-tiles of 128 tokens), variant "v4":
  PE:  score[tok,k] = x_aug.T @ cb_aug into PSUM (aug row folds -|e|^2/2;
       argmax score == argmin dist since ||x||^2 is constant per token).
       Two tiles share one 2-bank PSUM tensor.
  DVE: reduce_max over both tiles in one 3D op -> m[128,2]
       bias = -2^27 * m  (exact: power-of-two scale)
  ACT: onehot = tanh(2^27*score + bias) -> bf16 SBUF.  Exactly 0.0 at
       argmax positions (fma is exact for 2^27 multiples), negative
       elsewhere; bf16 preserves sign/zero exactly.
  DVE: MaxIndex(in_max=zeros, in_values=onehot) -> first index of 0.0
       == argmin-first. bf16 in_values for the fast path.
  GPSIMD: indirect DMA gather codebook rows by idx
  PE:  pair transpose [128tok, 2x64d] -> [128d, 128tok]
  ACT: copy PSUM->SBUF; DMA: store to out[b, :, tok_slice]
"""

import numpy as np

import concourse.bass as bass
import concourse.mybir as mybir
import concourse.tile as tile
from concourse.bass_utils import run_bass_kernel_spmd
from concourse.masks import make_identity

N_CORES = 8
B_FULL, D, H, W = 64, 64, 32, 32
N = H * W          # 1024 tokens per batch
K = 512            # codebook entries
B_CORE = B_FULL // N_CORES   # 8 batches per core
TOK_TILE = 128
TILES_PER_BATCH = N // TOK_TILE          # 8
N_TILES = B_CORE * TILES_PER_BATCH       # 64 per core

F32 = mybir.dt.float32
BF16 = mybir.dt.bfloat16
U16 = mybir.dt.uint16
U32 = mybir.dt.uint32

SCALE = float(2 ** 27)
MAX_SYNC_WAITS = 1

VARIANT = "v1"          # "v1" | "v4"
SPLIT_MM = False        # row-tiled split-K matmul (2 concurrent strips)
IDX_DTYPE = U16


def _split_excess_waits(nc, max_waits=MAX_SYNC_WAITS):
    """This container's walrus rejects >2 sync waits per instruction; move
    excess waits onto InstNoOp instructions inserted just before."""
    for f in nc.m.functions:
        for bb in f.blocks:
            new_list = []
            for inst in bb.instructions:
                si = inst.sync_info
                if si is not None and si.on_wait and len(si.on_wait) > max_waits:
                    waits = list(si.on_wait)
                    extra, keep = waits[:-max_waits], waits[-max_waits:]
                    for i, w in enumerate(extra):
                        nop = mybir.InstNoOp(name=f"{inst.name}-sw{i}", ins=[], outs=[])
                        nop.engine = inst.engine
                        nop.sync_info = mybir.SyncInfo(on_wait=[w], on_update=[])
                        new_list.append(nop)
                    si.on_wait = keep
                    new_list.append(inst)
                else:
                    new_list.append(inst)
            bb.instructions[:] = new_list


def _emit_mm(nc, score_ap, xt, cba):
    if SPLIT_MM:
        nc.tensor.matmul(score_ap, lhsT=xt[0:32, :], rhs=cba[0:32, :],
                         start=True, stop=False, tile_position=(0, 0))
        nc.tensor.matmul(score_ap, lhsT=xt[32:65, :], rhs=cba[32:65, :],
                         start=False, stop=True, tile_position=(32, 0))
    else:
        nc.tensor.matmul(score_ap, lhsT=xt[:], rhs=cba[:], start=True, stop=True)


def _emit_body_v4(nc, pools, x_aug, cb, out, cba, ident, zero8):
    sb, scp, pairp, ps, pst = pools
    for tp_idx in range(N_TILES // 2):
        t0, t1 = 2 * tp_idx, 2 * tp_idx + 1
        metas = []
        xts = []
        for t in (t0, t1):
            b, j = divmod(t, TILES_PER_BATCH)
            sl = slice(j * TOK_TILE, (j + 1) * TOK_TILE)
            metas.append((b, sl))
            xt = sb.tile([D + 1, TOK_TILE], F32, tag="xt")
            nc.sync.dma_start(out=xt[:], in_=x_aug[b, :, sl])
            xts.append(xt)

        # one 2-bank PSUM tensor holds both tiles' scores
        spair = ps.tile([TOK_TILE, 2, K], F32, tag="spair")
        for h in (0, 1):
            _emit_mm(nc, spair[:, h, :], xts[h], cba)

        # batched reduce over both tiles, then exact -2^27*m bias
        m2 = sb.tile([TOK_TILE, 2], F32, tag="m2")
        nc.vector.tensor_reduce(m2[:], spair[:], axis=mybir.AxisListType.X,
                                op=mybir.AluOpType.max)
        bias2 = sb.tile([TOK_TILE, 2], F32, tag="bias2")
        nc.vector.tensor_scalar_mul(bias2[:], m2[:], -SCALE)

        pr = pairp.tile([TOK_TILE, 128], F32, tag="pair")
        for h in (0, 1):
            oh = scp.tile([TOK_TILE, K], BF16, tag="oh")
            nc.scalar.activation(oh[:], spair[:, h, :],
                                 mybir.ActivationFunctionType.Tanh,
                                 bias=bias2[:, h:h + 1], scale=SCALE)
            idx8 = sb.tile([TOK_TILE, 8], IDX_DTYPE, tag="idx8")
            nc.vector.max_index(idx8[:], zero8[:], oh[:])
            if IDX_DTYPE == U32:
                idx_off = idx8[:, 0:1]
            else:
                idx1 = sb.tile([TOK_TILE, 1], U32, tag="idx1")
                nc.vector.tensor_copy(idx1[:], idx8[:, 0:1])
                idx_off = idx1[:]
            nc.gpsimd.indirect_dma_start(
                out=pr[:, h * D:(h + 1) * D],
                out_offset=None,
                in_=cb[:],
                in_offset=bass.IndirectOffsetOnAxis(ap=idx_off, axis=0),
            )

        tp = pst.tile([128, TOK_TILE], F32, tag="tp")
        nc.tensor.transpose(tp[:], pr[:], ident[:])
        tps = sb.tile([128, TOK_TILE], F32, tag="tps")
        nc.scalar.copy(tps[:], tp[:])
        for h, (bb_, sl_) in enumerate(metas):
            nc.sync.dma_start(out=out[bb_, :, sl_],
                              in_=tps[h * D:(h + 1) * D, :])


def _emit_body_v1(nc, pools, x_aug, cb, out, cba, ident, zero8):
    sb, scp, pairp, ps, pst = pools
    pr = None
    pr_meta = []
    for t in range(N_TILES):
        b, j = divmod(t, TILES_PER_BATCH)
        sl = slice(j * TOK_TILE, (j + 1) * TOK_TILE)
        xt = sb.tile([D + 1, TOK_TILE], F32, tag="xt")
        nc.sync.dma_start(out=xt[:], in_=x_aug[b, :, sl])
        score_ps = ps.tile([TOK_TILE, K], F32, tag="score_ps")
        _emit_mm(nc, score_ps[:], xt, cba)
        score = scp.tile([TOK_TILE, K], F32, tag="score")
        nc.scalar.copy(score[:], score_ps[:])
        mx8 = sb.tile([TOK_TILE, 8], F32, tag="mx8")
        nc.vector.max(mx8[:], score[:])
        idx8 = sb.tile([TOK_TILE, 8], U32, tag="idx8")
        nc.vector.max_index(idx8[:], mx8[:], score[:])
        if t % 2 == 0:
            pr = pairp.tile([TOK_TILE, 128], F32, tag="pair")
            pr_meta = []
        pr_meta.append((b, sl))
        nc.gpsimd.indirect_dma_start(
            out=pr[:, (t % 2) * D:(t % 2 + 1) * D],
            out_offset=None,
            in_=cb[:],
            in_offset=bass.IndirectOffsetOnAxis(ap=idx8[:, 0:1], axis=0),
        )
        if t % 2 == 1:
            tp = pst.tile([128, TOK_TILE], F32, tag="tp")
            nc.tensor.transpose(tp[:], pr[:], ident[:])
            tps = sb.tile([128, TOK_TILE], F32, tag="tps")
            nc.scalar.copy(tps[:], tp[:])
            for h, (bb_, sl_) in enumerate(pr_meta):
                nc.sync.dma_start(out=out[bb_, :, sl_],
                                  in_=tps[h * D:(h + 1) * D, :])


def build_nc(reps=1):
    nc = bass.Bass()
    x_aug = nc.dram_tensor("x_aug", [B_CORE, D + 1, N], F32, kind="ExternalInput")
    cb_aug = nc.dram_tensor("cb_aug", [D + 1, K], F32, kind="ExternalInput")
    cb = nc.dram_tensor("cb", [K, D], F32, kind="ExternalInput")
    out = nc.dram_tensor("out", [B_CORE, D, N], F32, kind="ExternalOutput")

    with tile.TileContext(nc) as tc:
        with (
            tc.tile_pool(name="const", bufs=1) as constp,
            tc.tile_pool(name="sbuf", bufs=8) as sb,
            tc.tile_pool(name="score", bufs=4) as scp,
            tc.tile_pool(name="pair", bufs=3) as pairp,
            tc.tile_pool(name="psum", bufs=4, space="PSUM") as ps,
            tc.tile_pool(name="psumt", bufs=3, space="PSUM") as pst,
        ):
            ident = constp.tile([128, 128], F32)
            make_identity(nc, ident[:])
            cba = constp.tile([D + 1, K], F32)
            nc.sync.dma_start(out=cba[:], in_=cb_aug[:])
            zero8 = constp.tile([128, 8], BF16 if VARIANT == "v4" else F32)
            nc.gpsimd.memset(zero8[:], 0)

            pools = (sb, scp, pairp, ps, pst)
            body = _emit_body_v4 if VARIANT == "v4" else _emit_body_v1
            for _rep in range(reps):
                body(nc, pools, x_aug, cb, out, cba, ident, zero8)

    _split_excess_waits(nc)
    return nc


_NC_CACHE = None


def _get_nc():
    global _NC_CACHE
    if _NC_CACHE is None:
        _NC_CACHE = build_nc()
    return _NC_CACHE


def prep_inputs(x, codebook):
    """Host-side prep: shard x over batch, augment with a ones-row; build
    cb_aug = [codebook.T ; -|e|^2/2]."""
    x = np.asarray(x, dtype=np.float32).reshape(B_FULL, D, N)
    codebook = np.asarray(codebook, dtype=np.float32)
    e2 = (codebook.astype(np.float64) ** 2).sum(-1).astype(np.float32)
    cb_aug = np.ascontiguousarray(np.concatenate(
        [codebook.T, (-0.5 * e2)[None, :]], axis=0).astype(np.float32))
    cb = np.ascontiguousarray(codebook)

    in_maps = []
    for c in range(N_CORES):
        xs = x[c * B_CORE:(c + 1) * B_CORE]                      # [8, 64, 1024]
        ones = np.ones((B_CORE, 1, N), dtype=np.float32)
        x_aug = np.ascontiguousarray(np.concatenate([xs, ones], axis=1))
        in_maps.append({"x_aug": x_aug, "cb_aug": cb_aug, "cb": cb})
    return in_maps


def kernel(x, codebook):
    nc = _get_nc()
    in_maps = prep_inputs(x, codebook)
    res = run_bass_kernel_spmd(nc, in_maps, core_ids=list(range(N_CORES)))
    out = np.concatenate([r["out"] for r in res.results], axis=0)
    return out.reshape(B_FULL, D, H, W)


if __name__ == "__main__":
    rng = np.random.default_rng(0)
    x = rng.standard_normal((B_FULL, D, H, W)).astype(np.float32)
    cbk = rng.standard_normal((K, D)).astype(np.float32)
    got = kernel(x, cbk)
    flat = x.reshape(B_FULL, D, N).transpose(0, 2, 1)
    dist = ((flat.astype(np.float64) ** 2).sum(-1, keepdims=True)
            - 2.0 * flat.astype(np.float64) @ cbk.T.astype(np.float64)
            + (cbk.astype(np.float64) ** 2).sum(-1))
    idx = dist.argmin(-1)
    exp = cbk[idx].transpose(0, 2, 1).reshape(B_FULL, D, H, W)
    err = np.linalg.norm(got - exp) / np.linalg.norm(exp)
    print("rel err vs numpy:", err)
    nflip = (got != exp).reshape(B_FULL, D, N).any(axis=1).sum()
    print("mismatched token count:", int(nflip))


# revision 5
# speedup vs baseline: 4.4796x; 1.5625x over previous
"""VQ codebook (nearest-codebook-entry) kernel for Trainium2, 8 NeuronCores.

Problem: x (64, 64, 32, 32) f32, codebook (512, 64) f32.
  flat = x.transpose -> (b, n=1024, d=64) tokens
  idx = argmin_k ||flat - codebook[k]||^2   (first-min, as jnp.argmin)
  out = codebook[idx] transposed back to (64, 64, 32, 32)

Sharding: data-parallel over batch (8 batches per core x 8 cores), codebook
replicated.

Per-core pipeline (64 token-tiles of 128 tokens), variant "v4":
  PE:  score[tok,k] = x_aug.T @ cb_aug into PSUM (aug row folds -|e|^2/2;
       argmax score == argmin dist since ||x||^2 is constant per token).
       Two tiles share one 2-bank PSUM tensor.
  DVE: reduce_max over both tiles in one 3D op -> m[128,2]
       bias = -2^27 * m  (exact: power-of-two scale)
  ACT: onehot = tanh(2^27*score + bias) -> bf16 SBUF.  Exactly 0.0 at
       argmax positions (fma is exact for 2^27 multiples), negative
       elsewhere; bf16 preserves sign/zero exactly.
  DVE: MaxIndex(in_max=zeros, in_values=onehot) -> first index of 0.0
       == argmin-first. bf16 in_values for the fast path.
  GPSIMD: indirect DMA gather codebook rows by idx
  PE:  pair transpose [128tok, 2x64d] -> [128d, 128tok]
  ACT: copy PSUM->SBUF; DMA: store to out[b, :, tok_slice]
"""

import numpy as np

import concourse.bass as bass
import concourse.mybir as mybir
import concourse.tile as tile
from concourse.bass_utils import run_bass_kernel_spmd
from concourse.masks import make_identity

N_CORES = 8
B_FULL, D, H, W = 64, 64, 32, 32
N = H * W          # 1024 tokens per batch
K = 512            # codebook entries
B_CORE = B_FULL // N_CORES   # 8 batches per core
TOK_TILE = 128
TILES_PER_BATCH = N // TOK_TILE          # 8
N_TILES = B_CORE * TILES_PER_BATCH       # 64 per core

F32 = mybir.dt.float32
BF16 = mybir.dt.bfloat16
U16 = mybir.dt.uint16
U32 = mybir.dt.uint32

SCALE = float(2 ** 27)
MAX_SYNC_WAITS = 1

VARIANT = "v1"          # "v1" | "v4"
SPLIT_MM = False        # row-tiled split-K matmul (2 concurrent strips)
IDX_DTYPE = U16


def _split_excess_waits(nc, max_waits=MAX_SYNC_WAITS):
    """This container's walrus rejects >2 sync waits per instruction; move
    excess waits onto InstNoOp instructions inserted just before."""
    for f in nc.m.functions:
        for bb in f.blocks:
            new_list = []
            for inst in bb.instructions:
                si = inst.sync_info
                if si is not None and si.on_wait and len(si.on_wait) > max_waits:
                    waits = list(si.on_wait)
                    extra, keep = waits[:-max_waits], waits[-max_waits:]
                    for i, w in enumerate(extra):
                        nop = mybir.InstNoOp(name=f"{inst.name}-sw{i}", ins=[], outs=[])
                        nop.engine = inst.engine
                        nop.sync_info = mybir.SyncInfo(on_wait=[w], on_update=[])
                        new_list.append(nop)
                    si.on_wait = keep
                    new_list.append(inst)
                else:
                    new_list.append(inst)
            bb.instructions[:] = new_list


def _emit_mm(nc, score_ap, xt, cba):
    if SPLIT_MM:
        # xt/cba live in a [97, .] layout: contract rows 0:32 at partitions
        # 0:32 (array strips 0), rows 32:65 at partitions 64:97 (strips 2-3)
        # so both row-tiles run concurrently and accumulate into one bank.
        nc.tensor.matmul(score_ap, lhsT=xt[0:32, :], rhs=cba[0:32, :],
                         start=True, stop=False, tile_position=(0, 0))
        nc.tensor.matmul(score_ap, lhsT=xt[64:97, :], rhs=cba[64:97, :],
                         start=False, stop=True, tile_position=(64, 0))
    else:
        nc.tensor.matmul(score_ap, lhsT=xt[:], rhs=cba[:], start=True, stop=True)


def _load_x_tile(nc, sb, x_aug, b, sl):
    if SPLIT_MM:
        xt = sb.tile([97, TOK_TILE], F32, tag="xt")
        nc.sync.dma_start(out=xt[0:32, :], in_=x_aug[b, 0:32, sl])
        nc.sync.dma_start(out=xt[64:97, :], in_=x_aug[b, 32:65, sl])
    else:
        xt = sb.tile([D + 1, TOK_TILE], F32, tag="xt")
        nc.sync.dma_start(out=xt[:], in_=x_aug[b, :, sl])
    return xt


def _emit_body_v4(nc, pools, x_aug, cb, out, cba, ident, zero8):
    sb, scp, pairp, ps, pst = pools
    for tp_idx in range(N_TILES // 2):
        t0, t1 = 2 * tp_idx, 2 * tp_idx + 1
        metas = []
        xts = []
        for t in (t0, t1):
            b, j = divmod(t, TILES_PER_BATCH)
            sl = slice(j * TOK_TILE, (j + 1) * TOK_TILE)
            metas.append((b, sl))
            xt = sb.tile([D + 1, TOK_TILE], F32, tag="xt")
            nc.sync.dma_start(out=xt[:], in_=x_aug[b, :, sl])
            xts.append(xt)

        # one 2-bank PSUM tensor holds both tiles' scores
        spair = ps.tile([TOK_TILE, 2, K], F32, tag="spair")
        for h in (0, 1):
            _emit_mm(nc, spair[:, h, :], xts[h], cba)

        # batched reduce over both tiles, then exact -2^27*m bias
        m2 = sb.tile([TOK_TILE, 2], F32, tag="m2")
        nc.vector.tensor_reduce(m2[:], spair[:], axis=mybir.AxisListType.X,
                                op=mybir.AluOpType.max)
        bias2 = sb.tile([TOK_TILE, 2], F32, tag="bias2")
        nc.vector.tensor_scalar_mul(bias2[:], m2[:], -SCALE)

        pr = pairp.tile([TOK_TILE, 128], F32, tag="pair")
        for h in (0, 1):
            oh = scp.tile([TOK_TILE, K], BF16, tag="oh")
            nc.scalar.activation(oh[:], spair[:, h, :],
                                 mybir.ActivationFunctionType.Tanh,
                                 bias=bias2[:, h:h + 1], scale=SCALE)
            idx8 = sb.tile([TOK_TILE, 8], IDX_DTYPE, tag="idx8")
            nc.vector.max_index(idx8[:], zero8[:], oh[:])
            if IDX_DTYPE == U32:
                idx_off = idx8[:, 0:1]
            else:
                idx1 = sb.tile([TOK_TILE, 1], U32, tag="idx1")
                nc.vector.tensor_copy(idx1[:], idx8[:, 0:1])
                idx_off = idx1[:]
            nc.gpsimd.indirect_dma_start(
                out=pr[:, h * D:(h + 1) * D],
                out_offset=None,
                in_=cb[:],
                in_offset=bass.IndirectOffsetOnAxis(ap=idx_off, axis=0),
            )

        tp = pst.tile([128, TOK_TILE], F32, tag="tp")
        nc.tensor.transpose(tp[:], pr[:], ident[:])
        tps = sb.tile([128, TOK_TILE], F32, tag="tps")
        nc.scalar.copy(tps[:], tp[:])
        for h, (bb_, sl_) in enumerate(metas):
            nc.sync.dma_start(out=out[bb_, :, sl_],
                              in_=tps[h * D:(h + 1) * D, :])


def _emit_body_v1(nc, pools, x_aug, cb, out, cba, ident, zero8):
    sb, scp, pairp, ps, pst = pools
    pr = None
    pr_meta = []
    for t in range(N_TILES):
        b, j = divmod(t, TILES_PER_BATCH)
        sl = slice(j * TOK_TILE, (j + 1) * TOK_TILE)
        xt = _load_x_tile(nc, sb, x_aug, b, sl)
        score_ps = ps.tile([TOK_TILE, K], F32, tag="score_ps")
        _emit_mm(nc, score_ps[:], xt, cba)
        score = scp.tile([TOK_TILE, K], F32, tag="score")
        nc.scalar.copy(score[:], score_ps[:])
        mx8 = sb.tile([TOK_TILE, 8], F32, tag="mx8")
        nc.vector.max(mx8[:], score[:])
        idx8 = sb.tile([TOK_TILE, 8], U32, tag="idx8")
        nc.vector.max_index(idx8[:], mx8[:], score[:])
        if t % 2 == 0:
            pr = pairp.tile([TOK_TILE, 128], F32, tag="pair")
            pr_meta = []
        pr_meta.append((b, sl))
        nc.gpsimd.indirect_dma_start(
            out=pr[:, (t % 2) * D:(t % 2 + 1) * D],
            out_offset=None,
            in_=cb[:],
            in_offset=bass.IndirectOffsetOnAxis(ap=idx8[:, 0:1], axis=0),
        )
        if t % 2 == 1:
            tp = pst.tile([128, TOK_TILE], F32, tag="tp")
            nc.tensor.transpose(tp[:], pr[:], ident[:])
            tps = sb.tile([128, TOK_TILE], F32, tag="tps")
            nc.scalar.copy(tps[:], tp[:])
            for h, (bb_, sl_) in enumerate(pr_meta):
                nc.sync.dma_start(out=out[bb_, :, sl_],
                                  in_=tps[h * D:(h + 1) * D, :])


def build_nc(reps=1):
    nc = bass.Bass()
    x_aug = nc.dram_tensor("x_aug", [B_CORE, D + 1, N], F32, kind="ExternalInput")
    cb_aug = nc.dram_tensor("cb_aug", [D + 1, K], F32, kind="ExternalInput")
    cb = nc.dram_tensor("cb", [K, D], F32, kind="ExternalInput")
    out = nc.dram_tensor("out", [B_CORE, D, N], F32, kind="ExternalOutput")

    with tile.TileContext(nc) as tc:
        with (
            tc.tile_pool(name="const", bufs=1) as constp,
            tc.tile_pool(name="sbuf", bufs=8) as sb,
            tc.tile_pool(name="score", bufs=4) as scp,
            tc.tile_pool(name="pair", bufs=3) as pairp,
            tc.tile_pool(name="psum", bufs=4, space="PSUM") as ps,
            tc.tile_pool(name="psumt", bufs=3, space="PSUM") as pst,
        ):
            ident = constp.tile([128, 128], F32)
            make_identity(nc, ident[:])
            if SPLIT_MM:
                cba = constp.tile([97, K], F32)
                nc.sync.dma_start(out=cba[0:32, :], in_=cb_aug[0:32, :])
                nc.sync.dma_start(out=cba[64:97, :], in_=cb_aug[32:65, :])
            else:
                cba = constp.tile([D + 1, K], F32)
                nc.sync.dma_start(out=cba[:], in_=cb_aug[:])
            zero8 = constp.tile([128, 8], BF16 if VARIANT == "v4" else F32)
            nc.gpsimd.memset(zero8[:], 0)

            pools = (sb, scp, pairp, ps, pst)
            body = _emit_body_v4 if VARIANT == "v4" else _emit_body_v1
            for _rep in range(reps):
                body(nc, pools, x_aug, cb, out, cba, ident, zero8)

    _split_excess_waits(nc)
    return nc


_NC_CACHE = None


def _get_nc():
    global _NC_CACHE
    if _NC_CACHE is None:
        _NC_CACHE = build_nc()
    return _NC_CACHE


def prep_inputs(x, codebook):
    """Host-side prep: shard x over batch, augment with a ones-row; build
    cb_aug = [codebook.T ; -|e|^2/2]."""
    x = np.asarray(x, dtype=np.float32).reshape(B_FULL, D, N)
    codebook = np.asarray(codebook, dtype=np.float32)
    e2 = (codebook.astype(np.float64) ** 2).sum(-1).astype(np.float32)
    cb_aug = np.ascontiguousarray(np.concatenate(
        [codebook.T, (-0.5 * e2)[None, :]], axis=0).astype(np.float32))
    cb = np.ascontiguousarray(codebook)

    in_maps = []
    for c in range(N_CORES):
        xs = x[c * B_CORE:(c + 1) * B_CORE]                      # [8, 64, 1024]
        ones = np.ones((B_CORE, 1, N), dtype=np.float32)
        x_aug = np.ascontiguousarray(np.concatenate([xs, ones], axis=1))
        in_maps.append({"x_aug": x_aug, "cb_aug": cb_aug, "cb": cb})
    return in_maps


def kernel(x, codebook):
    nc = _get_nc()
    in_maps = prep_inputs(x, codebook)
    res = run_bass_kernel_spmd(nc, in_maps, core_ids=list(range(N_CORES)))
    out = np.concatenate([r["out"] for r in res.results], axis=0)
    return out.reshape(B_FULL, D, H, W)


if __name__ == "__main__":
    rng = np.random.default_rng(0)
    x = rng.standard_normal((B_FULL, D, H, W)).astype(np.float32)
    cbk = rng.standard_normal((K, D)).astype(np.float32)
    got = kernel(x, cbk)
    flat = x.reshape(B_FULL, D, N).transpose(0, 2, 1)
    dist = ((flat.astype(np.float64) ** 2).sum(-1, keepdims=True)
            - 2.0 * flat.astype(np.float64) @ cbk.T.astype(np.float64)
            + (cbk.astype(np.float64) ** 2).sum(-1))
    idx = dist.argmin(-1)
    exp = cbk[idx].transpose(0, 2, 1).reshape(B_FULL, D, H, W)
    err = np.linalg.norm(got - exp) / np.linalg.norm(exp)
    print("rel err vs numpy:", err)
    nflip = (got != exp).reshape(B_FULL, D, N).any(axis=1).sum()
    print("mismatched token count:", int(nflip))
